# revision 6
# baseline (speedup 1.0000x reference)
"""NNUE HalfKA feature-transformer forward kernel for 8 trn2 NeuronCores.

Strategy (data-parallel over batch, hint-compliant):
  - Host folds the virtual-feature table into the main table:
        W[f, :] = ft_w.T[f, :] + fft_w.T[f % 768, :]          [49152, 1024]
    (valid because vboard @ fft_w.T is linear in the scatter-adds).
  - Each core owns 512 batch rows (B=4096 / 8). Its 512*32 nnz per
    perspective are packed into chunks of 128. Per chunk the device:
      * indirect-DMA gathers 128 table rows -> SBUF tile G [128, 1024]
      * builds an assignment matrix L[nnz, row] = value * (rowid == iota)
      * PE matmul P[row, :] += L.T @ G accumulated over chunks in PSUM
    giving the per-row feature sums for a 128-row group.
  - Epilogue per 128-row group: add bias, clip to [0,1], multiply by the
    matching half of out_w, reduce along features, add perspectives,
    sigmoid (out_b folded into the activation bias), DMA out.
  - Host concatenates the 8 per-core [512, 1] outputs.
"""

import sys

import numpy as np

for _p in ("/opt/trn_rl_repo",):
    if _p not in sys.path:
        sys.path.insert(0, _p)

B = 4096
NNZ_PER = 32
FT_IN = 49152
VIRT = 768
FT_OUT = 1024
N_CORES = 8
ROWS_PER_CORE = B // N_CORES  # 512
P = 128
GROUPS = ROWS_PER_CORE // P  # 4
NHALF = FT_OUT // 512  # 2 psum-bank halves


def _pack_streams(indices, values, core):
    """Pack one perspective's nnz for `core` into 128-wide chunks grouped by
    128-row group. Returns (feat[P,K], rowid[P,K], val[P,K], K_per_group)."""
    rows = np.asarray(indices[:, 0], dtype=np.int64)
    feats = np.asarray(indices[:, 1], dtype=np.int64)
    vals = np.asarray(values, dtype=np.float32)
    lo = core * ROWS_PER_CORE
    sel = (rows >= lo) & (rows < lo + ROWS_PER_CORE)
    r = (rows[sel] - lo).astype(np.int32)
    f = feats[sel].astype(np.int32)
    v = vals[sel]
    per_group = []
    for g in range(GROUPS):
        m = (r // P) == g
        per_group.append((f[m], r[m], v[m]))
    return per_group


def _pad_concat(per_group, K_per_group):
    """Pad each group's stream to K_per_group[g]*P entries and lay out as
    [P, sum(K)] column-per-chunk arrays."""
    f_cols, r_cols, v_cols = [], [], []
    for g, (f, r, v) in enumerate(per_group):
        n = K_per_group[g] * P
        pad = n - len(f)
        assert pad >= 0
        f = np.concatenate([f, np.zeros(pad, np.int32)])
        r = np.concatenate([r, np.full(pad, g * P, np.int32)])
        v = np.concatenate([v, np.zeros(pad, np.float32)])
        f_cols.append(f.reshape(-1, P).T)
        r_cols.append(r.reshape(-1, P).T)
        v_cols.append(v.reshape(-1, P).T)
    feat = np.ascontiguousarray(np.concatenate(f_cols, axis=1), dtype=np.int32)
    rowid = np.ascontiguousarray(
        np.concatenate(r_cols, axis=1).astype(np.float32)
    )
    val = np.ascontiguousarray(np.concatenate(v_cols, axis=1), dtype=np.float32)
    return feat, rowid, val


def _build_nc(K_stm, K_nstm, out_b_val):
    import concourse.bacc as bacc
    import concourse.bass as bass
    import concourse.mybir as mybir
    import concourse.tile as tile

    f32 = mybir.dt.float32
    i32 = mybir.dt.int32

    Ktot_stm = sum(K_stm)
    Ktot_nstm = sum(K_nstm)

    nc = bacc.Bacc(
        "TRN2",
        target_bir_lowering=False,
        debug=False,
        num_devices=N_CORES,
    )

    wt = nc.dram_tensor("wt", [FT_IN, FT_OUT], f32, kind="ExternalInput").ap()
    bias_d = nc.dram_tensor("bias", [P, FT_OUT], f32, kind="ExternalInput").ap()
    ow_stm_d = nc.dram_tensor("ow_stm", [P, FT_OUT], f32, kind="ExternalInput").ap()
    ow_nstm_d = nc.dram_tensor("ow_nstm", [P, FT_OUT], f32, kind="ExternalInput").ap()
    iota_d = nc.dram_tensor("iota", [P, ROWS_PER_CORE], f32, kind="ExternalInput").ap()
    feat_stm_d = nc.dram_tensor("feat_stm", [P, Ktot_stm], i32, kind="ExternalInput").ap()
    feat_nstm_d = nc.dram_tensor("feat_nstm", [P, Ktot_nstm], i32, kind="ExternalInput").ap()
    rowid_stm_d = nc.dram_tensor("rowid_stm", [P, Ktot_stm], f32, kind="ExternalInput").ap()
    rowid_nstm_d = nc.dram_tensor("rowid_nstm", [P, Ktot_nstm], f32, kind="ExternalInput").ap()
    val_stm_d = nc.dram_tensor("val_stm", [P, Ktot_stm], f32, kind="ExternalInput").ap()
    val_nstm_d = nc.dram_tensor("val_nstm", [P, Ktot_nstm], f32, kind="ExternalInput").ap()
    out_d = nc.dram_tensor("out", [ROWS_PER_CORE, 1], f32, kind="ExternalOutput").ap()

    persp_cfg = {
        "stm": (K_stm, feat_stm_d, rowid_stm_d, val_stm_d, Ktot_stm, ow_stm_d),
        "nstm": (K_nstm, feat_nstm_d, rowid_nstm_d, val_nstm_d, Ktot_nstm, ow_nstm_d),
    }

    with tile.TileContext(nc) as tc:
        from contextlib import ExitStack

        with ExitStack() as ctx:
            cpool = ctx.enter_context(tc.tile_pool(name="consts", bufs=1))
            gpool = ctx.enter_context(tc.tile_pool(name="gather", bufs=10))
            lpool = ctx.enter_context(tc.tile_pool(name="lmat", bufs=8))
            hpool = ctx.enter_context(tc.tile_pool(name="hidden", bufs=2))
            rpool = ctx.enter_context(tc.tile_pool(name="reduce", bufs=4))
            ppool = ctx.enter_context(tc.tile_pool(name="psum", bufs=2, space="PSUM"))

            # --- load constants / per-core packed streams into SBUF ---
            bias_sb = cpool.tile([P, FT_OUT], f32, tag="bias")
            nc.sync.dma_start(bias_sb[:], bias_d[:])
            iota_sb = cpool.tile([P, ROWS_PER_CORE], f32, tag="iota")
            nc.sync.dma_start(iota_sb[:], iota_d[:])

            sb = {}
            for name, (K, feat_d, rowid_d, val_d, Ktot, ow_d) in persp_cfg.items():
                feat_sb = cpool.tile([P, Ktot], i32, tag=f"feat_{name}")
                nc.sync.dma_start(feat_sb[:], feat_d[:])
                rowid_sb = cpool.tile([P, Ktot], f32, tag=f"rowid_{name}")
                nc.sync.dma_start(rowid_sb[:], rowid_d[:])
                val_sb = cpool.tile([P, Ktot], f32, tag=f"val_{name}")
                nc.sync.dma_start(val_sb[:], val_d[:])
                ow_sb = cpool.tile([P, FT_OUT], f32, tag=f"ow_{name}")
                nc.sync.dma_start(ow_sb[:], ow_d[:])
                sb[name] = (feat_sb, rowid_sb, val_sb, ow_sb)

            stage = cpool.tile([P, GROUPS], f32, tag="stage")
            outb_sb = cpool.tile([P, 1], f32, tag="outb")
            nc.vector.memset(outb_sb[:], float(out_b_val))

            base = {"stm": 0, "nstm": 0}
            for g in range(GROUPS):
                r_parts = {}
                for name, (K, feat_d, rowid_d, val_d, Ktot, ow_d) in persp_cfg.items():
                    feat_sb, rowid_sb, val_sb, ow_sb = sb[name]
                    Kg = K[g]
                    psum = [
                        ppool.tile(
                            [P, 512], f32, tag=f"ps_{name}{h}", name=f"ps_{name}{h}"
                        )
                        for h in range(NHALF)
                    ]
                    for k in range(Kg):
                        c = base[name] + k
                        G = gpool.tile([P, FT_OUT], f32, tag="G")
                        nc.gpsimd.indirect_dma_start(
                            out=G[:],
                            out_offset=None,
                            in_=wt[:],
                            in_offset=bass.IndirectOffsetOnAxis(
                                ap=feat_sb[:, c : c + 1], axis=0
                            ),
                        )
                        E = lpool.tile([P, P], f32, tag="E")
                        nc.vector.tensor_tensor(
                            out=E[:],
                            in0=rowid_sb[:, c : c + 1].to_broadcast([P, P]),
                            in1=iota_sb[:, g * P : (g + 1) * P],
                            op=mybir.AluOpType.is_equal,
                        )
                        L = lpool.tile([P, P], f32, tag="L")
                        nc.vector.tensor_tensor(
                            out=L[:],
                            in0=E[:],
                            in1=val_sb[:, c : c + 1].to_broadcast([P, P]),
                            op=mybir.AluOpType.mult,
                        )
                        for h in range(NHALF):
                            nc.tensor.matmul(
                                out=psum[h][:],
                                lhsT=L[:],
                                rhs=G[:, h * 512 : (h + 1) * 512],
                                start=(k == 0),
                                stop=(k == Kg - 1),
                            )
                    base[name] += Kg

                    # epilogue: bias + clip + dot(out_w half) + reduce
                    H = hpool.tile([P, FT_OUT], f32, tag=f"H_{name}")
                    for h in range(NHALF):
                        nc.vector.tensor_add(
                            out=H[:, h * 512 : (h + 1) * 512],
                            in0=psum[h][:],
                            in1=bias_sb[:, h * 512 : (h + 1) * 512],
                        )
                    nc.vector.tensor_scalar(
                        out=H[:],
                        in0=H[:],
                        scalar1=0.0,
                        scalar2=1.0,
                        op0=mybir.AluOpType.max,
                        op1=mybir.AluOpType.min,
                    )
                    nc.vector.tensor_mul(out=H[:], in0=H[:], in1=ow_sb[:])
                    r = rpool.tile([P, 1], f32, tag=f"r_{name}")
                    nc.vector.reduce_sum(r[:], H[:], axis=mybir.AxisListType.X)
                    r_parts[name] = r

                s = rpool.tile([P, 1], f32, tag="s")
                nc.vector.tensor_add(out=s[:], in0=r_parts["stm"][:], in1=r_parts["nstm"][:])
                nc.scalar.activation(
                    stage[:, g : g + 1],
                    s[:],
                    mybir.ActivationFunctionType.Sigmoid,
                    bias=outb_sb[:, :1],
                )
                nc.sync.dma_start(out_d[g * P : (g + 1) * P, :], stage[:, g : g + 1])

    nc.compile()
    return nc


LAST_RESULTS = None


def kernel(
    stm_indices,
    nstm_indices,
    values,
    buckets,
    ft_w,
    ft_b,
    fft_w,
    fft_b,
    out_w,
    out_b,
    _trace=False,
    _tmpdir=None,
):
    global LAST_RESULTS
    from concourse.bass_utils import run_bass_kernel_spmd

    ft_w = np.asarray(ft_w, dtype=np.float32)
    fft_w = np.asarray(fft_w, dtype=np.float32)

    # Fold virtual features into the main table: W[f] = ft_w.T[f] + fft_w.T[f%768]
    W = np.ascontiguousarray(ft_w.T)  # [49152, 1024]
    W = W.reshape(FT_IN // VIRT, VIRT, FT_OUT)
    W = W + np.ascontiguousarray(fft_w.T)[None, :, :]
    W = np.ascontiguousarray(W.reshape(FT_IN, FT_OUT), dtype=np.float32)

    bias_rep = np.ascontiguousarray(
        np.broadcast_to(
            (np.asarray(ft_b, np.float32) + np.asarray(fft_b, np.float32))[None, :],
            (P, FT_OUT),
        )
    )
    ow = np.asarray(out_w, np.float32).reshape(-1)
    ow_stm_rep = np.ascontiguousarray(np.broadcast_to(ow[None, :FT_OUT], (P, FT_OUT)))
    ow_nstm_rep = np.ascontiguousarray(np.broadcast_to(ow[None, FT_OUT:], (P, FT_OUT)))
    iota = np.ascontiguousarray(
        np.broadcast_to(
            np.arange(ROWS_PER_CORE, dtype=np.float32)[None, :], (P, ROWS_PER_CORE)
        )
    )

    # pack per-core nnz streams
    packed = {"stm": [], "nstm": []}
    for core in range(N_CORES):
        packed["stm"].append(_pack_streams(stm_indices, values, core))
        packed["nstm"].append(_pack_streams(nstm_indices, values, core))

    K = {}
    for name in ("stm", "nstm"):
        K[name] = [
            max(
                -(-len(packed[name][core][g][0]) // P)
                for core in range(N_CORES)
            )
            for g in range(GROUPS)
        ]

    in_maps = []
    for core in range(N_CORES):
        f_s, r_s, v_s = _pad_concat(packed["stm"][core], K["stm"])
        f_n, r_n, v_n = _pad_concat(packed["nstm"][core], K["nstm"])
        in_maps.append(
            {
                "wt": W,
                "bias": bias_rep,
                "ow_stm": ow_stm_rep,
                "ow_nstm": ow_nstm_rep,
                "iota": iota,
                "feat_stm": f_s,
                "rowid_stm": r_s,
                "val_stm": v_s,
                "feat_nstm": f_n,
                "rowid_nstm": r_n,
                "val_nstm": v_n,
            }
        )

    nc = _build_nc(K["stm"], K["nstm"], float(np.asarray(out_b).reshape(-1)[0]))

    res = run_bass_kernel_spmd(
        nc,
        in_maps,
        core_ids=list(range(N_CORES)),
        trace=_trace,
        tmpdir=_tmpdir,
    )
    LAST_RESULTS = res
    out = np.concatenate([res.results[c]["out"] for c in range(N_CORES)], axis=0)
    return np.ascontiguousarray(out, dtype=np.float32)


if __name__ == "__main__":
    # smoke test with tiny random data through the reference-free path
    rng = np.random.default_rng(0)
    rows = np.repeat(np.arange(B, dtype=np.int64), NNZ_PER)
    inputs = {
        "stm_indices": np.stack(
            [rows, rng.integers(0, FT_IN, B * NNZ_PER).astype(np.int64)], axis=1
        ),
        "nstm_indices": np.stack(
            [rows, rng.integers(0, FT_IN, B * NNZ_PER).astype(np.int64)], axis=1
        ),
        "values": np.ones(B * NNZ_PER, np.float32),
        "buckets": np.zeros(B, np.int64),
        "ft_w": rng.normal(size=(FT_OUT, FT_IN)).astype(np.float32) * 0.02,
        "ft_b": rng.normal(size=(FT_OUT,)).astype(np.float32) * 0.02,
        "fft_w": rng.normal(size=(FT_OUT, VIRT)).astype(np.float32) * 0.02,
        "fft_b": rng.normal(size=(FT_OUT,)).astype(np.float32) * 0.02,
        "out_w": rng.normal(size=(1, 2 * FT_OUT)).astype(np.float32) * 0.02,
        "out_b": rng.normal(size=(1,)).astype(np.float32) * 0.02,
    }
    out = kernel(**inputs)
    print("kernel out", out.shape, out.dtype, out[:4, 0])


# revision 10
# speedup vs baseline: 1.2980x; 1.2980x over previous
"""NNUE HalfKA feature-transformer forward kernel for 8 trn2 NeuronCores.

Strategy (data-parallel over batch, hint-compliant):
  - Host folds the virtual-feature table into the main table:
        W[f, :] = ft_w.T[f, :] + fft_w.T[f % 768, :]          [49152, 1024]
    (valid because vboard @ fft_w.T is linear in the scatter-adds).
  - Each core owns 512 batch rows (B=4096 / 8). Its 512*32 nnz per
    perspective are packed into chunks of 128. Per chunk the device:
      * indirect-DMA gathers 128 table rows -> SBUF tile G [128, 1024]
      * builds an assignment matrix L[nnz, row] = value * (rowid == iota)
      * PE matmul P[row, :] += L.T @ G accumulated over chunks in PSUM
    giving the per-row feature sums for a 128-row group.
  - Epilogue per 128-row group: add bias, clip to [0,1], multiply by the
    matching half of out_w, reduce along features, add perspectives,
    sigmoid (out_b folded into the activation bias), DMA out.
  - Host concatenates the 8 per-core [512, 1] outputs.
"""

import sys

import numpy as np

for _p in ("/opt/trn_rl_repo",):
    if _p not in sys.path:
        sys.path.insert(0, _p)

B = 4096
NNZ_PER = 32
FT_IN = 49152
VIRT = 768
FT_OUT = 1024
N_CORES = 8
ROWS_PER_CORE = B // N_CORES  # 512
P = 128
GROUPS = ROWS_PER_CORE // P  # 4
NHALF = FT_OUT // 512  # 2 psum-bank halves


def _pack_streams(indices, values, core):
    """Pack one perspective's nnz for `core` into 128-wide chunks grouped by
    128-row group. Returns (feat[P,K], rowid[P,K], val[P,K], K_per_group)."""
    rows = np.asarray(indices[:, 0], dtype=np.int64)
    feats = np.asarray(indices[:, 1], dtype=np.int64)
    vals = np.asarray(values, dtype=np.float32)
    lo = core * ROWS_PER_CORE
    sel = (rows >= lo) & (rows < lo + ROWS_PER_CORE)
    r = (rows[sel] - lo).astype(np.int32)
    f = feats[sel].astype(np.int32)
    v = vals[sel]
    per_group = []
    for g in range(GROUPS):
        m = (r // P) == g
        per_group.append((f[m], r[m], v[m]))
    return per_group


def _pad_concat(per_group, K_per_group):
    """Pad each group's stream to K_per_group[g]*P entries and lay out as
    [P, sum(K)] column-per-chunk arrays."""
    f_cols, r_cols, v_cols = [], [], []
    for g, (f, r, v) in enumerate(per_group):
        n = K_per_group[g] * P
        pad = n - len(f)
        assert pad >= 0
        f = np.concatenate([f, np.zeros(pad, np.int32)])
        r = np.concatenate([r, np.full(pad, g * P, np.int32)])
        v = np.concatenate([v, np.zeros(pad, np.float32)])
        f_cols.append(f.reshape(-1, P).T)
        r_cols.append(r.reshape(-1, P).T)
        v_cols.append(v.reshape(-1, P).T)
    feat = np.ascontiguousarray(np.concatenate(f_cols, axis=1), dtype=np.int32)
    rowid = np.ascontiguousarray(
        np.concatenate(r_cols, axis=1).astype(np.float32)
    )
    val = np.ascontiguousarray(np.concatenate(v_cols, axis=1), dtype=np.float32)
    return feat, rowid, val


def _build_nc(K_stm, K_nstm, out_b_val):
    import concourse.bacc as bacc
    import concourse.bass as bass
    import concourse.mybir as mybir
    import concourse.tile as tile

    f32 = mybir.dt.float32
    f16 = mybir.dt.float16
    i32 = mybir.dt.int32

    Ktot_stm = sum(K_stm)
    Ktot_nstm = sum(K_nstm)

    nc = bacc.Bacc(
        "TRN2",
        target_bir_lowering=False,
        debug=False,
        num_devices=N_CORES,
    )

    wt = nc.dram_tensor("wt", [FT_IN, FT_OUT], f16, kind="ExternalInput").ap()
    bias_d = nc.dram_tensor("bias", [P, FT_OUT], f32, kind="ExternalInput").ap()
    ow_stm_d = nc.dram_tensor("ow_stm", [P, FT_OUT], f32, kind="ExternalInput").ap()
    ow_nstm_d = nc.dram_tensor("ow_nstm", [P, FT_OUT], f32, kind="ExternalInput").ap()
    iota_d = nc.dram_tensor("iota", [P, ROWS_PER_CORE], f32, kind="ExternalInput").ap()
    feat_stm_d = nc.dram_tensor("feat_stm", [P, Ktot_stm], i32, kind="ExternalInput").ap()
    feat_nstm_d = nc.dram_tensor("feat_nstm", [P, Ktot_nstm], i32, kind="ExternalInput").ap()
    rowid_stm_d = nc.dram_tensor("rowid_stm", [P, Ktot_stm], f32, kind="ExternalInput").ap()
    rowid_nstm_d = nc.dram_tensor("rowid_nstm", [P, Ktot_nstm], f32, kind="ExternalInput").ap()
    val_stm_d = nc.dram_tensor("val_stm", [P, Ktot_stm], f32, kind="ExternalInput").ap()
    val_nstm_d = nc.dram_tensor("val_nstm", [P, Ktot_nstm], f32, kind="ExternalInput").ap()
    out_d = nc.dram_tensor("out", [ROWS_PER_CORE, 1], f32, kind="ExternalOutput").ap()

    persp_cfg = {
        "stm": (K_stm, feat_stm_d, rowid_stm_d, val_stm_d, Ktot_stm, ow_stm_d),
        "nstm": (K_nstm, feat_nstm_d, rowid_nstm_d, val_nstm_d, Ktot_nstm, ow_nstm_d),
    }

    with tile.TileContext(nc) as tc:
        from contextlib import ExitStack

        with ExitStack() as ctx:
            cpool = ctx.enter_context(tc.tile_pool(name="consts", bufs=1))
            gpool = ctx.enter_context(tc.tile_pool(name="gather", bufs=10))
            lpool = ctx.enter_context(tc.tile_pool(name="lmat", bufs=8))
            hpool = ctx.enter_context(tc.tile_pool(name="hidden", bufs=2))
            rpool = ctx.enter_context(tc.tile_pool(name="reduce", bufs=4))
            ppool = ctx.enter_context(tc.tile_pool(name="psum", bufs=2, space="PSUM"))

            # --- load constants / per-core packed streams into SBUF ---
            bias_sb = cpool.tile([P, FT_OUT], f32, tag="bias")
            nc.sync.dma_start(bias_sb[:], bias_d[:])
            iota_sb = cpool.tile([P, ROWS_PER_CORE], f32, tag="iota")
            nc.sync.dma_start(iota_sb[:], iota_d[:])

            sb = {}
            for name, (K, feat_d, rowid_d, val_d, Ktot, ow_d) in persp_cfg.items():
                feat_sb = cpool.tile([P, Ktot], i32, tag=f"feat_{name}")
                nc.sync.dma_start(feat_sb[:], feat_d[:])
                rowid_sb = cpool.tile([P, Ktot], f32, tag=f"rowid_{name}")
                nc.sync.dma_start(rowid_sb[:], rowid_d[:])
                val_sb = cpool.tile([P, Ktot], f32, tag=f"val_{name}")
                nc.sync.dma_start(val_sb[:], val_d[:])
                ow_sb = cpool.tile([P, FT_OUT], f32, tag=f"ow_{name}")
                nc.sync.dma_start(ow_sb[:], ow_d[:])
                sb[name] = (feat_sb, rowid_sb, val_sb, ow_sb)

            stage = cpool.tile([P, GROUPS], f32, tag="stage")
            outb_sb = cpool.tile([P, 1], f32, tag="outb")
            nc.vector.memset(outb_sb[:], float(out_b_val))

            base = {"stm": 0, "nstm": 0}
            for g in range(GROUPS):
                r_parts = {}
                for name, (K, feat_d, rowid_d, val_d, Ktot, ow_d) in persp_cfg.items():
                    feat_sb, rowid_sb, val_sb, ow_sb = sb[name]
                    Kg = K[g]
                    psum = [
                        ppool.tile(
                            [P, 512], f32, tag=f"ps_{name}{h}", name=f"ps_{name}{h}"
                        )
                        for h in range(NHALF)
                    ]
                    for k in range(Kg):
                        c = base[name] + k
                        G = gpool.tile([P, FT_OUT], f16, tag="G")
                        nc.gpsimd.indirect_dma_start(
                            out=G[:],
                            out_offset=None,
                            in_=wt[:],
                            in_offset=bass.IndirectOffsetOnAxis(
                                ap=feat_sb[:, c : c + 1], axis=0
                            ),
                        )
                        # L[nnz, m] = (iota[m] == rowid[nnz]) * value[nnz]
                        L = lpool.tile([P, P], f16, tag="L")
                        nc.vector.tensor_scalar(
                            out=L[:],
                            in0=iota_sb[:, g * P : (g + 1) * P],
                            scalar1=rowid_sb[:, c : c + 1],
                            scalar2=val_sb[:, c : c + 1],
                            op0=mybir.AluOpType.is_equal,
                            op1=mybir.AluOpType.mult,
                        )
                        for h in range(NHALF):
                            nc.tensor.matmul(
                                out=psum[h][:],
                                lhsT=L[:],
                                rhs=G[:, h * 512 : (h + 1) * 512],
                                start=(k == 0),
                                stop=(k == Kg - 1),
                            )
                    base[name] += Kg

                    # epilogue: bias + clip + dot(out_w half) + reduce
                    H = hpool.tile([P, FT_OUT], f32, tag=f"H_{name}")
                    for h in range(NHALF):
                        nc.vector.tensor_add(
                            out=H[:, h * 512 : (h + 1) * 512],
                            in0=psum[h][:],
                            in1=bias_sb[:, h * 512 : (h + 1) * 512],
                        )
                    nc.vector.tensor_scalar(
                        out=H[:],
                        in0=H[:],
                        scalar1=0.0,
                        scalar2=1.0,
                        op0=mybir.AluOpType.max,
                        op1=mybir.AluOpType.min,
                    )
                    nc.vector.tensor_mul(out=H[:], in0=H[:], in1=ow_sb[:])
                    r = rpool.tile([P, 1], f32, tag=f"r_{name}")
                    nc.vector.reduce_sum(r[:], H[:], axis=mybir.AxisListType.X)
                    r_parts[name] = r

                s = rpool.tile([P, 1], f32, tag="s")
                nc.vector.tensor_add(out=s[:], in0=r_parts["stm"][:], in1=r_parts["nstm"][:])
                nc.scalar.activation(
                    stage[:, g : g + 1],
                    s[:],
                    mybir.ActivationFunctionType.Sigmoid,
                    bias=outb_sb[:, :1],
                )
                nc.sync.dma_start(out_d[g * P : (g + 1) * P, :], stage[:, g : g + 1])

    nc.compile()
    return nc


LAST_RESULTS = None


def kernel(
    stm_indices,
    nstm_indices,
    values,
    buckets,
    ft_w,
    ft_b,
    fft_w,
    fft_b,
    out_w,
    out_b,
    _trace=False,
    _tmpdir=None,
):
    global LAST_RESULTS
    from concourse.bass_utils import run_bass_kernel_spmd

    ft_w = np.asarray(ft_w, dtype=np.float32)
    fft_w = np.asarray(fft_w, dtype=np.float32)

    # Fold virtual features into the main table: W[f] = ft_w.T[f] + fft_w.T[f%768]
    W = np.ascontiguousarray(ft_w.T)  # [49152, 1024]
    W = W.reshape(FT_IN // VIRT, VIRT, FT_OUT)
    W = W + np.ascontiguousarray(fft_w.T)[None, :, :]
    W = np.ascontiguousarray(W.reshape(FT_IN, FT_OUT), dtype=np.float16)

    bias_rep = np.ascontiguousarray(
        np.broadcast_to(
            (np.asarray(ft_b, np.float32) + np.asarray(fft_b, np.float32))[None, :],
            (P, FT_OUT),
        )
    )
    ow = np.asarray(out_w, np.float32).reshape(-1)
    ow_stm_rep = np.ascontiguousarray(np.broadcast_to(ow[None, :FT_OUT], (P, FT_OUT)))
    ow_nstm_rep = np.ascontiguousarray(np.broadcast_to(ow[None, FT_OUT:], (P, FT_OUT)))
    iota = np.ascontiguousarray(
        np.broadcast_to(
            np.arange(ROWS_PER_CORE, dtype=np.float32)[None, :], (P, ROWS_PER_CORE)
        )
    )

    # pack per-core nnz streams
    packed = {"stm": [], "nstm": []}
    for core in range(N_CORES):
        packed["stm"].append(_pack_streams(stm_indices, values, core))
        packed["nstm"].append(_pack_streams(nstm_indices, values, core))

    K = {}
    for name in ("stm", "nstm"):
        K[name] = [
            max(
                -(-len(packed[name][core][g][0]) // P)
                for core in range(N_CORES)
            )
            for g in range(GROUPS)
        ]

    in_maps = []
    for core in range(N_CORES):
        f_s, r_s, v_s = _pad_concat(packed["stm"][core], K["stm"])
        f_n, r_n, v_n = _pad_concat(packed["nstm"][core], K["nstm"])
        in_maps.append(
            {
                "wt": W,
                "bias": bias_rep,
                "ow_stm": ow_stm_rep,
                "ow_nstm": ow_nstm_rep,
                "iota": iota,
                "feat_stm": f_s,
                "rowid_stm": r_s,
                "val_stm": v_s,
                "feat_nstm": f_n,
                "rowid_nstm": r_n,
                "val_nstm": v_n,
            }
        )

    nc = _build_nc(K["stm"], K["nstm"], float(np.asarray(out_b).reshape(-1)[0]))

    res = run_bass_kernel_spmd(
        nc,
        in_maps,
        core_ids=list(range(N_CORES)),
        trace=_trace,
        tmpdir=_tmpdir,
    )
    LAST_RESULTS = res
    out = np.concatenate([res.results[c]["out"] for c in range(N_CORES)], axis=0)
    return np.ascontiguousarray(out, dtype=np.float32)


if __name__ == "__main__":
    # smoke test with tiny random data through the reference-free path
    rng = np.random.default_rng(0)
    rows = np.repeat(np.arange(B, dtype=np.int64), NNZ_PER)
    inputs = {
        "stm_indices": np.stack(
            [rows, rng.integers(0, FT_IN, B * NNZ_PER).astype(np.int64)], axis=1
        ),
        "nstm_indices": np.stack(
            [rows, rng.integers(0, FT_IN, B * NNZ_PER).astype(np.int64)], axis=1
        ),
        "values": np.ones(B * NNZ_PER, np.float32),
        "buckets": np.zeros(B, np.int64),
        "ft_w": rng.normal(size=(FT_OUT, FT_IN)).astype(np.float32) * 0.02,
        "ft_b": rng.normal(size=(FT_OUT,)).astype(np.float32) * 0.02,
        "fft_w": rng.normal(size=(FT_OUT, VIRT)).astype(np.float32) * 0.02,
        "fft_b": rng.normal(size=(FT_OUT,)).astype(np.float32) * 0.02,
        "out_w": rng.normal(size=(1, 2 * FT_OUT)).astype(np.float32) * 0.02,
        "out_b": rng.normal(size=(1,)).astype(np.float32) * 0.02,
    }
    out = kernel(**inputs)
    print("kernel out", out.shape, out.dtype, out[:4, 0])
